# revision 3
# baseline (speedup 1.0000x reference)
"""Batched Viterbi decode (CRF) on 8 Trainium2 NeuronCores — v5.

Device computes the bit-exact fp32 t1 value history; host backtracks.
Structure per step (8 seqs/core): per-seq bias-add (A_s + t1col) spread
across ACT/GPSIMD/DVE, per-seq PE transpose into PSUM, pair- or
quad-granular DVE max-reduce, GPSIMD quad em-adds into t1hist.

The per-reduce-group dependency chain is the step time; pairs shorten the
chain (reduce 462ns vs 690ns, 2 vs 4 serialized PE transposes) at the cost
of more DVE reduce instructions.

Env knobs:
  V5_RED   reduce group size: 2 (default) or 4
  V5_ADD   8-char engine string per seq: a=ACT bias-add, v=DVE
           tensor_scalar, g=GPSIMD tensor_tensor (default "agagagag")
  V5_EM    engine for quad em-adds: g=GPSIMD (default) or v=DVE
  V5_F32R  1 = bitcast PE transposes to float32r (default 0)
"""

import os
from contextlib import ExitStack

import numpy as np

S = 128
T = 2048
NS = 8
N_CORES = 8
B = NS * N_CORES

RED = int(os.environ.get("V5_RED", "2"))
ADD_ENG = os.environ.get("V5_ADD", "agagagag")
EM_ENG = os.environ.get("V5_EM", "g")
USE_F32R = os.environ.get("V5_F32R", "0") == "1"

_CACHE = {}


def _build_forward():
    import concourse.bacc as bacc
    import concourse.mybir as mybir
    import concourse.tile as tile

    F32 = mybir.dt.float32
    F32R = mybir.dt.float32r
    nc = bacc.Bacc("TRN2", num_devices=N_CORES)
    trans_in = nc.dram_tensor("transitions", [NS, S + 1, S], F32, kind="ExternalInput")
    em_in = nc.dram_tensor("emissions", [NS, T, S], F32, kind="ExternalInput")
    ident_in = nc.dram_tensor("identity", [S, S], F32, kind="ExternalInput")
    t1_out = nc.dram_tensor("t1hist", [S, T * NS], F32, kind="ExternalOutput")

    def tp(out_ap, in_ap, id_ap):
        if USE_F32R:
            nc.tensor.transpose(out_ap.bitcast(F32R), in_ap.bitcast(F32R),
                                id_ap.bitcast(F32R))
        else:
            nc.tensor.transpose(out_ap, in_ap, id_ap)

    NGRP = NS // RED  # reduce groups per step

    with ExitStack() as ctx:
        trans_sb = ctx.enter_context(nc.sbuf_tensor([S, NS * S], F32))
        em_cols = ctx.enter_context(nc.sbuf_tensor([S, T * NS], F32))
        t1hist = ctx.enter_context(nc.sbuf_tensor([S, T * NS], F32))
        ident = ctx.enter_context(nc.sbuf_tensor([S, S], F32))
        start_sb = ctx.enter_context(nc.sbuf_tensor([S, NS], F32))
        em0_sb = ctx.enter_context(nc.sbuf_tensor([S, NS], F32))
        # 2 quad-sized PSUM tensors x 3-deep rotation = 6 banks; prologue
        # stage psum uses the remaining 2.
        psum_pp = [
            [
                ctx.enter_context(nc.psum_tensor(f"pspp{k}g{g}", [S, 4 * S], F32))
                for g in range(2)
            ]
            for k in range(3)
        ]

        with tile.TileContext(nc) as tc, ExitStack() as pctx:
            sc_pool = pctx.enter_context(tc.tile_pool(name="scores", bufs=3))
            tmp_pool = pctx.enter_context(tc.tile_pool(name="tmp", bufs=3))
            stage_pool = pctx.enter_context(tc.tile_pool(name="stage", bufs=4))
            pst_pool = pctx.enter_context(tc.tile_pool(name="pst", bufs=2, space="PSUM"))

            # ---- prologue: transitions, identity, t1_0 ----
            for s in range(NS):
                nc.sync.dma_start(trans_sb[:, s * S:(s + 1) * S], trans_in[s, 0:S, :])
            nc.sync.dma_start(ident[:], ident_in[:])
            for s in range(NS):
                nc.sync.dma_start(
                    start_sb[:, s:s + 1], trans_in[s, S:S + 1, :].rearrange("o p -> p o")
                )
                nc.sync.dma_start(
                    em0_sb[:, s:s + 1], em_in[s, 0:1, :].rearrange("o p -> p o")
                )
            nc.vector.tensor_add(t1hist[:, 0:NS], start_sb[:], em0_sb[:])

            # ---- prologue: transpose emissions into em_cols[i, t*NS+s] ----
            for s in range(NS):
                for q in range(T // S // 4):
                    stage = stage_pool.tile([S, 4 * S], F32, tag="emstage")
                    pst = pst_pool.tile([S, 4 * S], F32, tag="empsum")
                    for k in range(4):
                        c = 4 * q + k
                        nc.sync.dma_start(
                            stage[:, k * S:(k + 1) * S], em_in[s, c * S:(c + 1) * S, :])
                        nc.tensor.transpose(
                            pst[:, k * S:(k + 1) * S], stage[:, k * S:(k + 1) * S],
                            ident[:])
                    dst = em_cols[:, 4 * q * S * NS + s: 4 * (q + 1) * S * NS: NS]
                    nc.scalar.copy(dst, pst[:])

            # ---- main DP loop ----
            def emit_add(s, t1col, dst):
                src = trans_sb[:, s * S:(s + 1) * S]
                e = ADD_ENG[s]
                if e == "a":
                    nc.scalar.activation(
                        dst, src, mybir.ActivationFunctionType.Identity,
                        bias=t1col, scale=1.0,
                    )
                elif e == "v":
                    nc.vector.tensor_scalar_add(dst, src, t1col)
                else:  # g
                    nc.gpsimd.tensor_add(
                        dst.rearrange("p (o i) -> p o i", i=S),
                        src.rearrange("p (o i) -> p o i", i=S),
                        t1col[:, :, None].to_broadcast([S, 1, S]))

            def emit_tiny(t, q):
                # t1hist[:, t*NS+4q : +4] = u + em  (quad granularity)
                lo = t * NS + 4 * q
                args = (t1hist[:, lo:lo + 4], tmp_cur[:, 4 * q:4 * q + 4],
                        em_cols[:, lo:lo + 4])
                if EM_ENG == "g":
                    nc.gpsimd.tensor_add(*args)
                else:
                    nc.vector.tensor_add(*args)

            tmp_cur = None

            def step(t):
                nonlocal tmp_cur
                base = (t - 1) * NS
                tmp_cur = tmp_pool.tile([S, NS], F32, tag="tmp")
                for grp in range(NGRP):
                    s0 = grp * RED
                    q = (s0 // 4)
                    pst = psum_pp[t % 3][q]
                    half = (s0 % 4)
                    sc_tiles = []
                    for sl in range(RED):
                        s = s0 + sl
                        sc = sc_pool.tile([S, S], F32, tag=f"sc{s}")
                        emit_add(s, t1hist[:, base + s:base + s + 1], sc[:])
                        sc_tiles.append(sc)
                    for sl in range(RED):
                        tp(pst[:, (half + sl) * S:(half + sl + 1) * S],
                           sc_tiles[sl][:], ident[:])
                    pg = pst[:, half * S:(half + RED) * S].rearrange(
                        "p (s i) -> p s i", i=S)
                    nc.vector.tensor_reduce(
                        tmp_cur[:, s0:s0 + RED], pg,
                        axis=mybir.AxisListType.X, op=mybir.AluOpType.max)
                    if (s0 + RED) % 4 == 0:
                        emit_tiny(t, (s0 + RED) // 4 - 1)

            CHUNK = 512
            for t in range(1, T):
                step(t)
                if t % CHUNK == 0:
                    lo = (t - CHUNK) * NS
                    nc.sync.dma_start(
                        t1_out[:, lo:t * NS], t1hist[:, lo:t * NS])

            lo = (T // CHUNK * CHUNK - CHUNK) * NS
            nc.sync.dma_start(t1_out[:, lo:], t1hist[:, lo:])

    nc.finalize()
    return nc


def _get_nc():
    if "nc" not in _CACHE:
        _CACHE["nc"] = _build_forward()
    return _CACHE["nc"]


def kernel(transitions, emissions, lengths):
    from concourse.bass_utils import run_bass_kernel_spmd

    transitions = np.ascontiguousarray(transitions, dtype=np.float32)
    emissions = np.ascontiguousarray(emissions, dtype=np.float32)
    lengths = np.asarray(lengths, dtype=np.int32)
    assert transitions.shape == (B, S + 1, S)
    assert emissions.shape == (B, T, S)

    nc = _get_nc()
    eye = np.eye(S, dtype=np.float32)
    in_maps = [
        {
            "transitions": transitions[c * NS:(c + 1) * NS],
            "emissions": emissions[c * NS:(c + 1) * NS],
            "identity": eye,
        }
        for c in range(N_CORES)
    ]
    res = run_bass_kernel_spmd(
        nc, in_maps, core_ids=list(range(N_CORES)),
        trace=bool(os.environ.get("VIT_TRACE")),
    )
    if os.environ.get("VIT_TRACE"):
        _CACHE["last_exec_time_ns"] = res.exec_time_ns
        _CACHE["last_res"] = res

    t1 = np.empty((B, T, S), dtype=np.float32)
    for c in range(N_CORES):
        t1[c * NS:(c + 1) * NS] = (
            res.results[c]["t1hist"].reshape(S, T, NS).transpose(2, 1, 0)
        )

    return _backtrack(transitions, emissions, lengths, t1)


def _backtrack(transitions, emissions, lengths, t1):
    """Reference-exact backtrack from the t1 value history."""
    trans = transitions[:, :S, :]
    nb = np.arange(B)
    z = np.zeros((B, T), dtype=np.int32)
    last = lengths - 1
    z_last = np.argmax(t1[nb, last, :], axis=1).astype(np.int32)
    ptr = z_last.copy()
    for t in range(int(last.max()), 0, -1):
        at_last = (t == last)
        if at_last.any():
            ptr = np.where(at_last, z_last, ptr)
        z[:, t] = np.where(t <= last, ptr, 0)
        col = (t1[:, t - 1, :] + trans[nb, :, ptr]) + emissions[nb, t, ptr][:, None]
        ptr_new = np.argmax(col, axis=1).astype(np.int32)
        ptr = np.where(t <= last, ptr_new, ptr)
    z[:, 0] = ptr
    return z


# revision 8
# speedup vs baseline: 1.2200x; 1.2200x over previous
"""Batched Viterbi decode (CRF) on 8 Trainium2 NeuronCores — v5.

Device computes the bit-exact fp32 t1 value history; host backtracks.
Structure per step (8 seqs/core): per-seq bias-add (A_s + t1col) spread
across ACT/GPSIMD/DVE, per-seq PE transpose into PSUM, pair- or
quad-granular DVE max-reduce, GPSIMD quad em-adds into t1hist.

The per-reduce-group dependency chain is the step time; pairs shorten the
chain (reduce 462ns vs 690ns, 2 vs 4 serialized PE transposes) at the cost
of more DVE reduce instructions.

Env knobs:
  V5_RED   reduce group size: 2 (default) or 4
  V5_ADD   8-char engine string per seq: a=ACT bias-add, v=DVE
           tensor_scalar, g=GPSIMD tensor_tensor (default "agagagag")
  V5_EM    engine for quad em-adds: g=GPSIMD (default) or v=DVE
  V5_F32R  1 = bitcast PE transposes to float32r (default 0)
"""

import os
from contextlib import ExitStack

import numpy as np

S = 128
T = 2048
NS = 8
N_CORES = 8
B = NS * N_CORES

RED = int(os.environ.get("V5_RED", "4"))
ADD_ENG = os.environ.get("V5_ADD", "aavaavaa")
EM_ENG = os.environ.get("V5_EM", "v")
# 0 = fp32 transposes; 2 = fp32r-typed transpose path (ident/sc/psum
# tensors declared float32r, engine reads/writes via fp32 bitcast)
F32R_MODE = int(os.environ.get("V5_F32R", "0"))

_CACHE = {}


def _build_forward():
    import concourse.bacc as bacc
    import concourse.mybir as mybir
    import concourse.tile as tile

    F32 = mybir.dt.float32
    F32R = mybir.dt.float32r
    nc = bacc.Bacc("TRN2", num_devices=N_CORES)
    trans_in = nc.dram_tensor("transitions", [NS, S + 1, S], F32, kind="ExternalInput")
    em_in = nc.dram_tensor("emissions", [NS, T, S], F32, kind="ExternalInput")
    ident_in = nc.dram_tensor("identity", [S, S], F32, kind="ExternalInput")
    t1_out = nc.dram_tensor("t1hist", [S, T * NS], F32, kind="ExternalOutput")

    NGRP = NS // RED  # reduce groups per step
    TPDT = F32R if F32R_MODE == 2 else F32

    with ExitStack() as ctx:
        trans_sb = ctx.enter_context(nc.sbuf_tensor([S, NS * S], F32))
        em_cols = ctx.enter_context(nc.sbuf_tensor([S, T * NS], F32))
        t1hist = ctx.enter_context(nc.sbuf_tensor([S, T * NS], F32))
        ident = ctx.enter_context(nc.sbuf_tensor([S, S], F32))
        start_sb = ctx.enter_context(nc.sbuf_tensor([S, NS], F32))
        em0_sb = ctx.enter_context(nc.sbuf_tensor([S, NS], F32))
        if F32R_MODE == 2:
            ident_r = ctx.enter_context(nc.sbuf_tensor([S, S], F32R))
        # 2 quad-sized PSUM tensors x 3-deep rotation = 6 banks; prologue
        # stage psum uses the remaining 2.
        psum_pp = [
            [
                ctx.enter_context(nc.psum_tensor(f"pspp{k}g{g}", [S, 4 * S], TPDT))
                for g in range(2)
            ]
            for k in range(3)
        ]

        with tile.TileContext(nc) as tc, ExitStack() as pctx:
            sc_pool = pctx.enter_context(tc.tile_pool(name="scores", bufs=3))
            tmp_pool = pctx.enter_context(tc.tile_pool(name="tmp", bufs=3))
            stage_pool = pctx.enter_context(tc.tile_pool(name="stage", bufs=4))
            pst_pool = pctx.enter_context(tc.tile_pool(name="pst", bufs=2, space="PSUM"))

            # ---- prologue: transitions, identity, t1_0 ----
            for s in range(NS):
                nc.sync.dma_start(trans_sb[:, s * S:(s + 1) * S], trans_in[s, 0:S, :])
            nc.sync.dma_start(ident[:], ident_in[:])
            if F32R_MODE == 2:
                nc.sync.dma_start(ident_r[:], ident_in[:].bitcast(F32R))
            for s in range(NS):
                nc.sync.dma_start(
                    start_sb[:, s:s + 1], trans_in[s, S:S + 1, :].rearrange("o p -> p o")
                )
                nc.sync.dma_start(
                    em0_sb[:, s:s + 1], em_in[s, 0:1, :].rearrange("o p -> p o")
                )
            nc.vector.tensor_add(t1hist[:, 0:NS], start_sb[:], em0_sb[:])

            # ---- prologue: transpose emissions into em_cols[i, t*NS+s] ----
            for s in range(NS):
                for q in range(T // S // 4):
                    stage = stage_pool.tile([S, 4 * S], F32, tag="emstage")
                    pst = pst_pool.tile([S, 4 * S], F32, tag="empsum")
                    for k in range(4):
                        c = 4 * q + k
                        nc.sync.dma_start(
                            stage[:, k * S:(k + 1) * S], em_in[s, c * S:(c + 1) * S, :])
                        nc.tensor.transpose(
                            pst[:, k * S:(k + 1) * S], stage[:, k * S:(k + 1) * S],
                            ident[:])
                    dst = em_cols[:, 4 * q * S * NS + s: 4 * (q + 1) * S * NS: NS]
                    nc.scalar.copy(dst, pst[:])

            # ---- main DP loop ----
            def emit_add(s, t1col, dst):
                src = trans_sb[:, s * S:(s + 1) * S]
                e = ADD_ENG[s]
                if e == "a":
                    nc.scalar.activation(
                        dst, src, mybir.ActivationFunctionType.Identity,
                        bias=t1col, scale=1.0,
                    )
                elif e == "v":
                    nc.vector.tensor_scalar_add(dst, src, t1col)
                else:  # g
                    nc.gpsimd.tensor_add(
                        dst.rearrange("p (o i) -> p o i", i=S),
                        src.rearrange("p (o i) -> p o i", i=S),
                        t1col[:, :, None].to_broadcast([S, 1, S]))

            def emit_tiny(t, q):
                # t1hist[:, t*NS+4q : +4] = u + em  (quad granularity)
                lo = t * NS + 4 * q
                args = (t1hist[:, lo:lo + 4], tmp_cur[:, 4 * q:4 * q + 4],
                        em_cols[:, lo:lo + 4])
                if EM_ENG == "g":
                    nc.gpsimd.tensor_add(*args)
                else:
                    nc.vector.tensor_add(*args)

            tmp_cur = None

            def step(t):
                nonlocal tmp_cur
                base = (t - 1) * NS
                tmp_cur = tmp_pool.tile([S, NS], F32, tag="tmp")
                for grp in range(NGRP):
                    s0 = grp * RED
                    q = (s0 // 4)
                    pst = psum_pp[t % 3][q]
                    half = (s0 % 4)
                    sc_tiles = []
                    for sl in range(RED):
                        s = s0 + sl
                        sc = sc_pool.tile([S, S], TPDT, tag=f"sc{s}")
                        dst = sc[:].bitcast(F32) if F32R_MODE == 2 else sc[:]
                        emit_add(s, t1hist[:, base + s:base + s + 1], dst)
                        sc_tiles.append(sc)
                    for sl in range(RED):
                        nc.tensor.transpose(
                            pst[:, (half + sl) * S:(half + sl + 1) * S],
                            sc_tiles[sl][:],
                            ident_r[:] if F32R_MODE == 2 else ident[:])
                    pgsrc = pst[:, half * S:(half + RED) * S]
                    if F32R_MODE == 2:
                        pgsrc = pgsrc.bitcast(F32)
                    pg = pgsrc.rearrange("p (s i) -> p s i", i=S)
                    nc.vector.tensor_reduce(
                        tmp_cur[:, s0:s0 + RED], pg,
                        axis=mybir.AxisListType.X, op=mybir.AluOpType.max)
                    if (s0 + RED) % 4 == 0:
                        emit_tiny(t, (s0 + RED) // 4 - 1)

            CHUNK = 512
            for t in range(1, T):
                step(t)
                if t % CHUNK == 0:
                    lo = (t - CHUNK) * NS
                    nc.sync.dma_start(
                        t1_out[:, lo:t * NS], t1hist[:, lo:t * NS])

            lo = (T // CHUNK * CHUNK - CHUNK) * NS
            nc.sync.dma_start(t1_out[:, lo:], t1hist[:, lo:])

    nc.finalize()
    return nc


def _get_nc():
    if "nc" not in _CACHE:
        _CACHE["nc"] = _build_forward()
    return _CACHE["nc"]


def kernel(transitions, emissions, lengths):
    from concourse.bass_utils import run_bass_kernel_spmd

    transitions = np.ascontiguousarray(transitions, dtype=np.float32)
    emissions = np.ascontiguousarray(emissions, dtype=np.float32)
    lengths = np.asarray(lengths, dtype=np.int32)
    assert transitions.shape == (B, S + 1, S)
    assert emissions.shape == (B, T, S)

    nc = _get_nc()
    eye = np.eye(S, dtype=np.float32)
    in_maps = [
        {
            "transitions": transitions[c * NS:(c + 1) * NS],
            "emissions": emissions[c * NS:(c + 1) * NS],
            "identity": eye,
        }
        for c in range(N_CORES)
    ]
    res = run_bass_kernel_spmd(
        nc, in_maps, core_ids=list(range(N_CORES)),
        trace=bool(os.environ.get("VIT_TRACE")),
    )
    if os.environ.get("VIT_TRACE"):
        _CACHE["last_exec_time_ns"] = res.exec_time_ns
        _CACHE["last_res"] = res

    t1 = np.empty((B, T, S), dtype=np.float32)
    for c in range(N_CORES):
        t1[c * NS:(c + 1) * NS] = (
            res.results[c]["t1hist"].reshape(S, T, NS).transpose(2, 1, 0)
        )

    return _backtrack(transitions, emissions, lengths, t1)


def _backtrack(transitions, emissions, lengths, t1):
    """Reference-exact backtrack from the t1 value history."""
    trans = transitions[:, :S, :]
    nb = np.arange(B)
    z = np.zeros((B, T), dtype=np.int32)
    last = lengths - 1
    z_last = np.argmax(t1[nb, last, :], axis=1).astype(np.int32)
    ptr = z_last.copy()
    for t in range(int(last.max()), 0, -1):
        at_last = (t == last)
        if at_last.any():
            ptr = np.where(at_last, z_last, ptr)
        z[:, t] = np.where(t <= last, ptr, 0)
        col = (t1[:, t - 1, :] + trans[nb, :, ptr]) + emissions[nb, t, ptr][:, None]
        ptr_new = np.argmax(col, axis=1).astype(np.int32)
        ptr = np.where(t <= last, ptr_new, ptr)
    z[:, 0] = ptr
    return z


# revision 10
# speedup vs baseline: 1.5087x; 1.2367x over previous
"""Batched Viterbi decode (CRF) on 8 Trainium2 NeuronCores — v5.

Device computes the bit-exact fp32 t1 value history; host backtracks.
Structure per step (8 seqs/core): per-seq bias-add (A_s + t1col) spread
across ACT/GPSIMD/DVE, per-seq PE transpose into PSUM, pair- or
quad-granular DVE max-reduce, GPSIMD quad em-adds into t1hist.

The per-reduce-group dependency chain is the step time; pairs shorten the
chain (reduce 462ns vs 690ns, 2 vs 4 serialized PE transposes) at the cost
of more DVE reduce instructions.

Env knobs:
  V5_RED   reduce group size: 2 (default) or 4
  V5_ADD   8-char engine string per seq: a=ACT bias-add, v=DVE
           tensor_scalar, g=GPSIMD tensor_tensor (default "agagagag")
  V5_EM    engine for quad em-adds: g=GPSIMD (default) or v=DVE
  V5_F32R  1 = bitcast PE transposes to float32r (default 0)
"""

import os
from contextlib import ExitStack

import numpy as np

S = 128
T = 2048
NS = 8
N_CORES = 8
B = NS * N_CORES

RED = int(os.environ.get("V5_RED", "4"))
ADD_ENG = os.environ.get("V5_ADD", "aagvaagv")
EM_ENG = os.environ.get("V5_EM", "v")
# 0 = fp32 transposes; 2 = fp32r-typed transpose path (ident/sc/psum
# tensors declared float32r, engine reads/writes via fp32 bitcast)
F32R_MODE = int(os.environ.get("V5_F32R", "0"))

_CACHE = {}


def _build_forward():
    import concourse.bacc as bacc
    import concourse.mybir as mybir
    import concourse.tile as tile

    F32 = mybir.dt.float32
    F32R = mybir.dt.float32r
    nc = bacc.Bacc("TRN2", num_devices=N_CORES)
    trans_in = nc.dram_tensor("transitions", [NS, S + 1, S], F32, kind="ExternalInput")
    em_in = nc.dram_tensor("emissions", [NS, T, S], F32, kind="ExternalInput")
    ident_in = nc.dram_tensor("identity", [S, S], F32, kind="ExternalInput")
    t1_out = nc.dram_tensor("t1hist", [S, T * NS], F32, kind="ExternalOutput")

    NGRP = NS // RED  # reduce groups per step
    TPDT = F32R if F32R_MODE == 2 else F32

    with ExitStack() as ctx:
        trans_sb = ctx.enter_context(nc.sbuf_tensor([S, NS * S], F32))
        em_cols = ctx.enter_context(nc.sbuf_tensor([S, T * NS], F32))
        t1hist = ctx.enter_context(nc.sbuf_tensor([S, T * NS], F32))
        ident = ctx.enter_context(nc.sbuf_tensor([S, S], F32))
        start_sb = ctx.enter_context(nc.sbuf_tensor([S, NS], F32))
        em0_sb = ctx.enter_context(nc.sbuf_tensor([S, NS], F32))
        if F32R_MODE == 2:
            ident_r = ctx.enter_context(nc.sbuf_tensor([S, S], F32R))
        # 2 quad-sized PSUM tensors x 3-deep rotation = 6 banks; prologue
        # stage psum uses the remaining 2.
        psum_pp = [
            [
                ctx.enter_context(nc.psum_tensor(f"pspp{k}g{g}", [S, 4 * S], TPDT))
                for g in range(2)
            ]
            for k in range(3)
        ]

        with tile.TileContext(nc) as tc, ExitStack() as pctx:
            sc_pool = pctx.enter_context(tc.tile_pool(name="scores", bufs=3))
            tmp_pool = pctx.enter_context(tc.tile_pool(name="tmp", bufs=1))
            stage_pool = pctx.enter_context(tc.tile_pool(name="stage", bufs=4))
            pst_pool = pctx.enter_context(tc.tile_pool(name="pst", bufs=2, space="PSUM"))

            # ---- prologue: transitions, identity, t1_0 ----
            for s in range(NS):
                nc.sync.dma_start(trans_sb[:, s * S:(s + 1) * S], trans_in[s, 0:S, :])
            nc.sync.dma_start(ident[:], ident_in[:])
            if F32R_MODE == 2:
                nc.sync.dma_start(ident_r[:], ident_in[:].bitcast(F32R))
            for s in range(NS):
                nc.sync.dma_start(
                    start_sb[:, s:s + 1], trans_in[s, S:S + 1, :].rearrange("o p -> p o")
                )
                nc.sync.dma_start(
                    em0_sb[:, s:s + 1], em_in[s, 0:1, :].rearrange("o p -> p o")
                )
            nc.vector.tensor_add(t1hist[:, 0:NS], start_sb[:], em0_sb[:])

            # ---- prologue: transpose emissions into em_cols[i, t*NS+s] ----
            for s in range(NS):
                for q in range(T // S // 4):
                    stage = stage_pool.tile([S, 4 * S], F32, tag="emstage")
                    pst = pst_pool.tile([S, 4 * S], F32, tag="empsum")
                    for k in range(4):
                        c = 4 * q + k
                        nc.sync.dma_start(
                            stage[:, k * S:(k + 1) * S], em_in[s, c * S:(c + 1) * S, :])
                        nc.tensor.transpose(
                            pst[:, k * S:(k + 1) * S], stage[:, k * S:(k + 1) * S],
                            ident[:])
                    dst = em_cols[:, 4 * q * S * NS + s: 4 * (q + 1) * S * NS: NS]
                    nc.scalar.copy(dst, pst[:])

            # ---- main DP loop ----
            def emit_add(s, t1col, dst):
                src = trans_sb[:, s * S:(s + 1) * S]
                e = ADD_ENG[s]
                if e == "a":
                    nc.scalar.activation(
                        dst, src, mybir.ActivationFunctionType.Identity,
                        bias=t1col, scale=1.0,
                    )
                elif e == "v":
                    nc.vector.tensor_scalar_add(dst, src, t1col)
                else:  # g
                    nc.gpsimd.tensor_add(
                        dst.rearrange("p (o i) -> p o i", i=S),
                        src.rearrange("p (o i) -> p o i", i=S),
                        t1col[:, :, None].to_broadcast([S, 1, S]))

            def emit_tiny(t, s0, tmp):
                # t1hist[:, t*NS+s0 : +RED] = u + em  (group granularity)
                lo = t * NS + s0
                args = (t1hist[:, lo:lo + RED], tmp[:],
                        em_cols[:, lo:lo + RED])
                if EM_ENG == "g":
                    nc.gpsimd.tensor_add(*args)
                else:
                    nc.vector.tensor_add(*args)

            def step(t):
                base = (t - 1) * NS
                for grp in range(NGRP):
                    s0 = grp * RED
                    q = (s0 // 4)
                    pst = psum_pp[t % 3][q]
                    half = (s0 % 4)
                    # emit ACT adds first: they serialize on the Scalar
                    # engine, so release them as early as possible
                    order = sorted(range(RED),
                                   key=lambda sl: ADD_ENG[s0 + sl] != "a")
                    sc_tiles = {}
                    for sl in order:
                        s = s0 + sl
                        sc = sc_pool.tile([S, S], TPDT, tag=f"sc{s}")
                        dst = sc[:].bitcast(F32) if F32R_MODE == 2 else sc[:]
                        emit_add(s, t1hist[:, base + s:base + s + 1], dst)
                        sc_tiles[sl] = sc
                    for sl in order:
                        nc.tensor.transpose(
                            pst[:, (half + sl) * S:(half + sl + 1) * S],
                            sc_tiles[sl][:],
                            ident_r[:] if F32R_MODE == 2 else ident[:])
                    pgsrc = pst[:, half * S:(half + RED) * S]
                    if F32R_MODE == 2:
                        pgsrc = pgsrc.bitcast(F32)
                    pg = pgsrc.rearrange("p (s i) -> p s i", i=S)
                    # single-buffered tmp shared by both groups: the WAR
                    # dependency forces the scheduler to keep each group's
                    # reduce->tiny adjacent on DVE instead of batching both
                    # reduces first (which puts group1's reduce on group0's
                    # critical path)
                    tmp = tmp_pool.tile([S, RED], F32, tag="u")
                    nc.vector.tensor_reduce(
                        tmp[:], pg,
                        axis=mybir.AxisListType.X, op=mybir.AluOpType.max)
                    emit_tiny(t, s0, tmp)

            CHUNK = 512
            for t in range(1, T):
                step(t)
                if t % CHUNK == 0:
                    lo = (t - CHUNK) * NS
                    nc.sync.dma_start(
                        t1_out[:, lo:t * NS], t1hist[:, lo:t * NS])

            lo = (T // CHUNK * CHUNK - CHUNK) * NS
            nc.sync.dma_start(t1_out[:, lo:], t1hist[:, lo:])

    nc.finalize()
    return nc


def _get_nc():
    if "nc" not in _CACHE:
        _CACHE["nc"] = _build_forward()
    return _CACHE["nc"]


def kernel(transitions, emissions, lengths):
    from concourse.bass_utils import run_bass_kernel_spmd

    transitions = np.ascontiguousarray(transitions, dtype=np.float32)
    emissions = np.ascontiguousarray(emissions, dtype=np.float32)
    lengths = np.asarray(lengths, dtype=np.int32)
    assert transitions.shape == (B, S + 1, S)
    assert emissions.shape == (B, T, S)

    nc = _get_nc()
    eye = np.eye(S, dtype=np.float32)
    in_maps = [
        {
            "transitions": transitions[c * NS:(c + 1) * NS],
            "emissions": emissions[c * NS:(c + 1) * NS],
            "identity": eye,
        }
        for c in range(N_CORES)
    ]
    res = run_bass_kernel_spmd(
        nc, in_maps, core_ids=list(range(N_CORES)),
        trace=bool(os.environ.get("VIT_TRACE")),
    )
    if os.environ.get("VIT_TRACE"):
        _CACHE["last_exec_time_ns"] = res.exec_time_ns
        _CACHE["last_res"] = res

    t1 = np.empty((B, T, S), dtype=np.float32)
    for c in range(N_CORES):
        t1[c * NS:(c + 1) * NS] = (
            res.results[c]["t1hist"].reshape(S, T, NS).transpose(2, 1, 0)
        )

    return _backtrack(transitions, emissions, lengths, t1)


def _backtrack(transitions, emissions, lengths, t1):
    """Reference-exact backtrack from the t1 value history."""
    trans = transitions[:, :S, :]
    nb = np.arange(B)
    z = np.zeros((B, T), dtype=np.int32)
    last = lengths - 1
    z_last = np.argmax(t1[nb, last, :], axis=1).astype(np.int32)
    ptr = z_last.copy()
    for t in range(int(last.max()), 0, -1):
        at_last = (t == last)
        if at_last.any():
            ptr = np.where(at_last, z_last, ptr)
        z[:, t] = np.where(t <= last, ptr, 0)
        col = (t1[:, t - 1, :] + trans[nb, :, ptr]) + emissions[nb, t, ptr][:, None]
        ptr_new = np.argmax(col, axis=1).astype(np.int32)
        ptr = np.where(t <= last, ptr_new, ptr)
    z[:, 0] = ptr
    return z


# revision 14
# speedup vs baseline: 1.5549x; 1.0306x over previous
"""Batched Viterbi decode (CRF) on 8 Trainium2 NeuronCores — v5.

Device computes the bit-exact fp32 t1 value history; host backtracks.
Structure per step (8 seqs/core): per-seq bias-add (A_s + t1col) spread
across ACT/GPSIMD/DVE, per-seq PE transpose into PSUM, pair- or
quad-granular DVE max-reduce, GPSIMD quad em-adds into t1hist.

The per-reduce-group dependency chain is the step time; pairs shorten the
chain (reduce 462ns vs 690ns, 2 vs 4 serialized PE transposes) at the cost
of more DVE reduce instructions.

Env knobs:
  V5_RED   reduce group size: 2 (default) or 4
  V5_ADD   8-char engine string per seq: a=ACT bias-add, v=DVE
           tensor_scalar, g=GPSIMD tensor_tensor (default "agagagag")
  V5_EM    engine for quad em-adds: g=GPSIMD (default) or v=DVE
  V5_F32R  1 = bitcast PE transposes to float32r (default 0)
"""

import os
from contextlib import ExitStack

import numpy as np

S = 128
T = 2048
NS = 8
N_CORES = 8
B = NS * N_CORES

RED = int(os.environ.get("V5_RED", "4"))
ADD_ENG = os.environ.get("V5_ADD", "aaggaagg")
EM_ENG = os.environ.get("V5_EM", "v")
# 0 = fp32 transposes; 2 = fp32r-typed transpose path (ident/sc/psum
# tensors declared float32r, engine reads/writes via fp32 bitcast)
F32R_MODE = int(os.environ.get("V5_F32R", "0"))

_CACHE = {}


def _build_forward():
    import concourse.bacc as bacc
    import concourse.mybir as mybir
    import concourse.tile as tile

    F32 = mybir.dt.float32
    F32R = mybir.dt.float32r
    nc = bacc.Bacc("TRN2", num_devices=N_CORES)
    trans_in = nc.dram_tensor("transitions", [NS, S + 1, S], F32, kind="ExternalInput")
    em_in = nc.dram_tensor("emissions", [NS, T, S], F32, kind="ExternalInput")
    ident_in = nc.dram_tensor("identity", [S, S], F32, kind="ExternalInput")
    t1_out = nc.dram_tensor("t1hist", [S, T * NS], F32, kind="ExternalOutput")

    NGRP = NS // RED  # reduce groups per step
    TPDT = F32R if F32R_MODE == 2 else F32

    with ExitStack() as ctx:
        trans_sb = ctx.enter_context(nc.sbuf_tensor([S, NS * S], F32))
        em_cols = ctx.enter_context(nc.sbuf_tensor([S, T * NS], F32))
        t1hist = ctx.enter_context(nc.sbuf_tensor([S, T * NS], F32))
        ident = ctx.enter_context(nc.sbuf_tensor([S, S], F32))
        start_sb = ctx.enter_context(nc.sbuf_tensor([S, NS], F32))
        em0_sb = ctx.enter_context(nc.sbuf_tensor([S, NS], F32))
        if F32R_MODE == 2:
            ident_r = ctx.enter_context(nc.sbuf_tensor([S, S], F32R))
        # 2 quad-sized PSUM tensors x 3-deep rotation = 6 banks; prologue
        # stage psum uses the remaining 2.
        psum_pp = [
            [
                ctx.enter_context(nc.psum_tensor(f"pspp{k}g{g}", [S, 4 * S], TPDT))
                for g in range(2)
            ]
            for k in range(3)
        ]

        with tile.TileContext(nc) as tc, ExitStack() as pctx:
            sc_pool = pctx.enter_context(tc.tile_pool(name="scores", bufs=3))
            tmp_pool = pctx.enter_context(tc.tile_pool(name="tmp", bufs=1))
            stage_pool = pctx.enter_context(tc.tile_pool(name="stage", bufs=4))
            pst_pool = pctx.enter_context(tc.tile_pool(name="pst", bufs=2, space="PSUM"))

            # ---- prologue: transitions, identity, t1_0 ----
            for s in range(NS):
                nc.sync.dma_start(trans_sb[:, s * S:(s + 1) * S], trans_in[s, 0:S, :])
            nc.sync.dma_start(ident[:], ident_in[:])
            if F32R_MODE == 2:
                nc.sync.dma_start(ident_r[:], ident_in[:].bitcast(F32R))
            for s in range(NS):
                nc.sync.dma_start(
                    start_sb[:, s:s + 1], trans_in[s, S:S + 1, :].rearrange("o p -> p o")
                )
                nc.sync.dma_start(
                    em0_sb[:, s:s + 1], em_in[s, 0:1, :].rearrange("o p -> p o")
                )
            nc.vector.tensor_add(t1hist[:, 0:NS], start_sb[:], em0_sb[:])

            # ---- prologue: transpose emissions into em_cols[i, t*NS+s] ----
            for s in range(NS):
                for q in range(T // S // 4):
                    stage = stage_pool.tile([S, 4 * S], F32, tag="emstage")
                    pst = pst_pool.tile([S, 4 * S], F32, tag="empsum")
                    for k in range(4):
                        c = 4 * q + k
                        nc.sync.dma_start(
                            stage[:, k * S:(k + 1) * S], em_in[s, c * S:(c + 1) * S, :])
                        nc.tensor.transpose(
                            pst[:, k * S:(k + 1) * S], stage[:, k * S:(k + 1) * S],
                            ident[:])
                    dst = em_cols[:, 4 * q * S * NS + s: 4 * (q + 1) * S * NS: NS]
                    nc.scalar.copy(dst, pst[:])

            # ---- main DP loop ----
            def emit_add(s, t1col, dst):
                src = trans_sb[:, s * S:(s + 1) * S]
                e = ADD_ENG[s]
                if e == "a":
                    nc.scalar.activation(
                        dst, src, mybir.ActivationFunctionType.Identity,
                        bias=t1col, scale=1.0,
                    )
                elif e == "v":
                    nc.vector.tensor_scalar_add(dst, src, t1col)
                else:  # g
                    nc.gpsimd.tensor_add(
                        dst.rearrange("p (o i) -> p o i", i=S),
                        src.rearrange("p (o i) -> p o i", i=S),
                        t1col[:, :, None].to_broadcast([S, 1, S]))

            def emit_tiny(t, s0, tmp):
                # t1hist[:, t*NS+s0 : +RED] = u + em  (group granularity)
                lo = t * NS + s0
                args = (t1hist[:, lo:lo + RED], tmp[:],
                        em_cols[:, lo:lo + RED])
                if EM_ENG == "g":
                    nc.gpsimd.tensor_add(*args)
                else:
                    nc.vector.tensor_add(*args)

            def step(t):
                base = (t - 1) * NS
                for grp in range(NGRP):
                    s0 = grp * RED
                    q = (s0 // 4)
                    pst = psum_pp[t % 3][q]
                    half = (s0 % 4)
                    # emit ACT adds first: they serialize on the Scalar
                    # engine, so release them as early as possible
                    order = sorted(range(RED),
                                   key=lambda sl: ADD_ENG[s0 + sl] != "a")
                    sc_tiles = {}
                    for sl in order:
                        s = s0 + sl
                        sc = sc_pool.tile([S, S], TPDT, tag=f"sc{s}")
                        dst = sc[:].bitcast(F32) if F32R_MODE == 2 else sc[:]
                        emit_add(s, t1hist[:, base + s:base + s + 1], dst)
                        sc_tiles[sl] = sc
                    # transposes in expected ready-time order: a slot's add
                    # finishes after the other adds queued before it on the
                    # same engine, so interleave engines by FIFO rank to
                    # avoid head-of-line blocking in the PE queue
                    nseen = {}
                    ranks = {}
                    for sl in order:
                        e = ADD_ENG[s0 + sl]
                        ranks[sl] = nseen.get(e, 0)
                        nseen[e] = ranks[sl] + 1
                    tp_order = sorted(order, key=lambda sl: ranks[sl])
                    for sl in tp_order:
                        nc.tensor.transpose(
                            pst[:, (half + sl) * S:(half + sl + 1) * S],
                            sc_tiles[sl][:],
                            ident_r[:] if F32R_MODE == 2 else ident[:])
                    pgsrc = pst[:, half * S:(half + RED) * S]
                    if F32R_MODE == 2:
                        pgsrc = pgsrc.bitcast(F32)
                    pg = pgsrc.rearrange("p (s i) -> p s i", i=S)
                    # single-buffered tmp shared by both groups: the WAR
                    # dependency forces the scheduler to keep each group's
                    # reduce->tiny adjacent on DVE instead of batching both
                    # reduces first (which puts group1's reduce on group0's
                    # critical path)
                    tmp = tmp_pool.tile([S, RED], F32, tag="u")
                    nc.vector.tensor_reduce(
                        tmp[:], pg,
                        axis=mybir.AxisListType.X, op=mybir.AluOpType.max)
                    emit_tiny(t, s0, tmp)

            CHUNK = 512
            for t in range(1, T):
                step(t)
                if t % CHUNK == 0:
                    lo = (t - CHUNK) * NS
                    nc.sync.dma_start(
                        t1_out[:, lo:t * NS], t1hist[:, lo:t * NS])

            lo = (T // CHUNK * CHUNK - CHUNK) * NS
            nc.sync.dma_start(t1_out[:, lo:], t1hist[:, lo:])

    nc.finalize()
    return nc


def _get_nc():
    if "nc" not in _CACHE:
        _CACHE["nc"] = _build_forward()
    return _CACHE["nc"]


def kernel(transitions, emissions, lengths):
    from concourse.bass_utils import run_bass_kernel_spmd

    transitions = np.ascontiguousarray(transitions, dtype=np.float32)
    emissions = np.ascontiguousarray(emissions, dtype=np.float32)
    lengths = np.asarray(lengths, dtype=np.int32)
    assert transitions.shape == (B, S + 1, S)
    assert emissions.shape == (B, T, S)

    nc = _get_nc()
    eye = np.eye(S, dtype=np.float32)
    in_maps = [
        {
            "transitions": transitions[c * NS:(c + 1) * NS],
            "emissions": emissions[c * NS:(c + 1) * NS],
            "identity": eye,
        }
        for c in range(N_CORES)
    ]
    res = run_bass_kernel_spmd(
        nc, in_maps, core_ids=list(range(N_CORES)),
        trace=bool(os.environ.get("VIT_TRACE")),
    )
    if os.environ.get("VIT_TRACE"):
        _CACHE["last_exec_time_ns"] = res.exec_time_ns
        _CACHE["last_res"] = res

    t1 = np.empty((B, T, S), dtype=np.float32)
    for c in range(N_CORES):
        t1[c * NS:(c + 1) * NS] = (
            res.results[c]["t1hist"].reshape(S, T, NS).transpose(2, 1, 0)
        )

    return _backtrack(transitions, emissions, lengths, t1)


def _backtrack(transitions, emissions, lengths, t1):
    """Reference-exact backtrack from the t1 value history."""
    trans = transitions[:, :S, :]
    nb = np.arange(B)
    z = np.zeros((B, T), dtype=np.int32)
    last = lengths - 1
    z_last = np.argmax(t1[nb, last, :], axis=1).astype(np.int32)
    ptr = z_last.copy()
    for t in range(int(last.max()), 0, -1):
        at_last = (t == last)
        if at_last.any():
            ptr = np.where(at_last, z_last, ptr)
        z[:, t] = np.where(t <= last, ptr, 0)
        col = (t1[:, t - 1, :] + trans[nb, :, ptr]) + emissions[nb, t, ptr][:, None]
        ptr_new = np.argmax(col, axis=1).astype(np.int32)
        ptr = np.where(t <= last, ptr_new, ptr)
    z[:, 0] = ptr
    return z
